# revision 21
# baseline (speedup 1.0000x reference)
"""Trainium2 Bass kernel for nn_AttentionModel (graph attention encoder + decoder).

Contract: kernel(**inputs) takes FULL unsharded numpy inputs (as produced by
reference.setup_inputs()) and returns the FULL [256, 100] float32 output.
Internally shards the batch (256) across 8 NeuronCores (32 each, pure data
parallel; weights replicated) and runs a fused Bass/Tile kernel per core.

Self-contained: hardcodes all shapes; no sibling imports.
"""

import sys

for _p in ("/opt/trn_rl_repo", "/opt/pypackages"):
    if _p not in sys.path:
        sys.path.append(_p)

import numpy as np
from contextlib import ExitStack

# --- static architecture constants ---
B, IH, IL, LH, E, FFH, NL = 256, 200, 6, 100, 256, 512, 2
G = IH + LH + 1  # 301
CLIP = 10.0
SCALE = 1.0 / 16.0  # 1/sqrt(E)
NCORES = 8
BPC = B // NCORES  # 32 batch elements per core

# g chunks over 301 nodes, e chunks over 256 features, f chunks over 512
GC = [(0, 128), (128, 256), (256, 301)]
GC2 = [(0, 128), (128, 256), (256, 302)]  # even-padded for fp32r matmuls
VN = 302  # even-padded moving width over the node axis
ECN = 2  # e chunks of 128
FCN = 4  # ff chunks of 128


# ----------------------------------------------------------------------------
# host-side weight packing
# ----------------------------------------------------------------------------
def _tf32(x):
    """Round fp32 array to tfloat32 (10 mantissa bits), round-to-nearest-even."""
    u = np.ascontiguousarray(x, np.float32).view(np.uint32)
    u = (u + 0x0FFF + ((u >> 13) & 1)) & np.uint32(0xFFFFE000)
    return u.view(np.float32)


def _pack_rows(m, nchunk):
    """[nchunk*128, N] -> [128, nchunk, N] with [:, k, :] = m[128k:128(k+1), :]"""
    return np.ascontiguousarray(
        np.stack([m[i * 128:(i + 1) * 128] for i in range(nchunk)], axis=1)
    ).astype(np.float32)


def _prep_weights(inp):
    w = {}
    w["wi1"] = inp["wi1"].astype(np.float32)          # [6, 32]
    w["wl1"] = inp["wl1"].astype(np.float32)          # [8, 32]
    w["wn1"] = inp["wn1"].astype(np.float32)          # [6, 32]
    w["b1r_i"] = inp["bi1"].reshape(1, 32).astype(np.float32)
    w["b1r_l"] = inp["bl1"].reshape(1, 32).astype(np.float32)
    w["b1r_n"] = inp["bn1"].reshape(1, 32).astype(np.float32)
    w["w2_i"] = _tf32(inp["wi2"])         # [32, 256]
    w["w2_l"] = _tf32(inp["wl2"])
    w["w2_n"] = _tf32(inp["wn2"])
    w["b2r_i"] = _tf32(inp["bi2"].reshape(1, E))
    w["b2r_l"] = _tf32(inp["bl2"].reshape(1, E))
    w["b2r_n"] = _tf32(inp["bn2"].reshape(1, E))
    for l in range(NL):
        # fold the 1/sqrt(E) attention scale into Wq
        w[f"wq{l}"] = _tf32(_pack_rows(inp["enc_wq"][l] * SCALE, 2))   # [128,2,256] lhsT chunks
        w[f"wk{l}"] = _tf32(_pack_rows(inp["enc_wk"][l], 2))
        w[f"wv{l}"] = _tf32(_pack_rows(inp["enc_wv"][l], 2))           # rhs chunks
        w[f"wo{l}"] = _tf32(_pack_rows(inp["enc_wo"][l], 2))           # rhs chunks
        w[f"wf1{l}"] = _tf32(_pack_rows(inp["enc_wf1"][l], 2))         # [128,2,512] lhsT chunks
        w[f"bf1{l}"] = np.ascontiguousarray(
            inp["enc_bf1"][l].reshape(4, 128).T
        ).astype(np.float32)                                    # [128, 4]
        w[f"wf2{l}"] = _tf32(_pack_rows(inp["enc_wf2"][l], 4))         # [128,4,256] rhs chunks
        w[f"bf2{l}"] = _tf32(inp["enc_bf2"][l].reshape(1, E))
    # decoder fused matrix: compat = h_leaf . (M @ ge), M = Wpn_E @ Wfc.T
    # lhsT for c = M @ ge is MT = M.T = Wfc @ Wpn_E.T ; fold 1/sqrt(E) here
    MT = (inp["w_fc"] @ inp["w_pn"][:, :E].T) * SCALE
    w["mt"] = _pack_rows(MT, 2)                                 # [128,2,256]
    return w


# ----------------------------------------------------------------------------
# numpy mirror of the device computation (for algebra validation)
# ----------------------------------------------------------------------------
def _numpy_mirror(observation, w):
    obs = observation.astype(np.float32)
    nb = obs.shape[0]
    out = np.zeros((nb, LH), np.float32)

    def lrelu(x):
        return np.maximum(x, 0.01 * x)

    def ln(x):
        m = x.mean(-1, keepdims=True)
        v = x.var(-1, keepdims=True)
        return (x - m) / np.sqrt(v + 1e-5)

    for b in range(nb):
        xT = obs[b, :, :9].T  # [9, 301]
        h = np.zeros((G, E), np.float32)
        z_i = xT[:6, :IH].T @ w["wi1"] + w["b1r_i"]
        z_l = xT[:8, IH:IH + LH].T @ w["wl1"] + w["b1r_l"]
        z_n = xT[:6, IH + LH:].T @ w["wn1"] + w["b1r_n"]
        h[:IH] = lrelu(z_i) @ w["w2_i"] + w["b2r_i"]
        h[IH:IH + LH] = lrelu(z_l) @ w["w2_l"] + w["b2r_l"]
        h[IH + LH:] = lrelu(z_n) @ w["w2_n"] + w["b2r_n"]

        for l in range(NL):
            wq = np.concatenate([w[f"wq{l}"][:, 0], w[f"wq{l}"][:, 1]], 0)
            wk = np.concatenate([w[f"wk{l}"][:, 0], w[f"wk{l}"][:, 1]], 0)
            wv = np.concatenate([w[f"wv{l}"][:, 0], w[f"wv{l}"][:, 1]], 0)
            wo = np.concatenate([w[f"wo{l}"][:, 0], w[f"wo{l}"][:, 1]], 0)
            wf1 = np.concatenate([w[f"wf1{l}"][:, 0], w[f"wf1{l}"][:, 1]], 0)
            wf2 = np.concatenate([w[f"wf2{l}"][:, k] for k in range(4)], 0)
            bf1 = w[f"bf1{l}"].T.reshape(-1)
            q = h @ wq  # already scaled by 1/16
            k = h @ wk
            v = h @ wv
            s = q @ k.T
            mx = s.max(-1, keepdims=True)
            e = np.exp(s - mx)
            rs = e.sum(-1, keepdims=True)
            o = (e @ v) / rs @ wo
            h = ln(h + o)
            f = np.maximum(h @ wf1 + bf1, 0.0) @ wf2 + w[f"bf2{l}"]
            h = ln(h + f)

        mask = obs[b, :, 8]
        trans = h * mask[:, None]
        ge = trans.sum(0)  # unnormalized
        MT = np.concatenate([w["mt"][:, 0], w["mt"][:, 1]], 0)
        c = MT.T @ ge  # [256]
        compat = trans[IH:IH + LH] @ c  # [100]
        vlen = mask.sum()
        logits = np.tanh(compat / vlen) * CLIP
        ee = np.exp(logits)
        p = ee / ee.sum()
        lv = obs[b, IH:IH + LH, 8]
        masked = p * lv + 1e-20
        out[b] = masked / masked.sum()
    return out


# ----------------------------------------------------------------------------
# the Bass/Tile kernel
# ----------------------------------------------------------------------------
def _build(bpc, dbg=False):
    import concourse.bass as bass
    import concourse.mybir as mybir
    import concourse.tile as tile
    from concourse import bacc
    from concourse.masks import make_identity

    f32 = mybir.dt.float32
    f32r = mybir.dt.float32r
    AF = mybir.ActivationFunctionType
    ALU = mybir.AluOpType
    AX = mybir.AxisListType

    def r(ap):
        return ap.bitcast(f32r)

    nc = bacc.Bacc(None, target_bir_lowering=False)

    obs = nc.declare_dram_parameter("obs", [bpc, G, 9], f32, isOutput=False)
    dp = {}
    dp["wi1"] = nc.declare_dram_parameter("wi1", [6, 32], f32, isOutput=False)
    dp["wl1"] = nc.declare_dram_parameter("wl1", [8, 32], f32, isOutput=False)
    dp["wn1"] = nc.declare_dram_parameter("wn1", [6, 32], f32, isOutput=False)
    F32_WEIGHTS = {"wi1", "wl1", "wn1", "b1r_i", "b1r_l", "b1r_n", "bf10", "bf11"}
    for t in "iln":
        dp[f"b1r_{t}"] = nc.declare_dram_parameter(f"b1r_{t}", [1, 32], f32, isOutput=False)
        dp[f"w2_{t}"] = nc.declare_dram_parameter(f"w2_{t}", [32, E], f32r, isOutput=False)
        dp[f"b2r_{t}"] = nc.declare_dram_parameter(f"b2r_{t}", [1, E], f32r, isOutput=False)
    for l in range(NL):
        for nm, shp in (
            (f"wq{l}", [128, 2, E]), (f"wk{l}", [128, 2, E]),
            (f"wv{l}", [128, 2, E]), (f"wo{l}", [128, 2, E]),
            (f"wf1{l}", [128, 2, FFH]), (f"bf1{l}", [128, 4]),
            (f"wf2{l}", [128, 4, E]), (f"bf2{l}", [1, E]),
        ):
            wdt = f32 if nm in F32_WEIGHTS else f32r
            dp[nm] = nc.declare_dram_parameter(nm, shp, wdt, isOutput=False)
    dp["mt"] = nc.declare_dram_parameter("mt", [128, 2, E], f32, isOutput=False)
    out_d = nc.declare_dram_parameter("out", [bpc, LH], f32, isOutput=True)
    if dbg:
        dbg_h0 = nc.declare_dram_parameter("dbg_h0", [128, 3, E], f32, isOutput=True)
        dbg_a = nc.declare_dram_parameter("dbg_a", [128, 3, 304], f32, isOutput=True)
        dbg_h1 = nc.declare_dram_parameter("dbg_h1", [128, 3, E], f32, isOutput=True)
        dbg_h2 = nc.declare_dram_parameter("dbg_h2", [128, 3, E], f32, isOutput=True)
        dbg_rinv = nc.declare_dram_parameter("dbg_rinv", [128, 3], f32, isOutput=True)
        dbg_z1 = nc.declare_dram_parameter("dbg_z1", [32, G], f32, isOutput=True)
        dbg_lr = nc.declare_dram_parameter("dbg_lr", [32, G], f32, isOutput=True)

    with tile.TileContext(nc) as tc, ExitStack() as ctx:
        const = ctx.enter_context(tc.tile_pool(name="const", bufs=1))
        st = ctx.enter_context(tc.tile_pool(name="st", bufs=2))
        sm = ctx.enter_context(tc.tile_pool(name="sm", bufs=3))
        ps = ctx.enter_context(tc.tile_pool(name="ps", bufs=6, space="PSUM"))

        # ---- constants / weights into SBUF ----
        ident = const.tile([128, 128], f32, tag="ident")
        make_identity(nc, ident)
        ident_r = const.tile([128, 128], f32r, tag="ident_r")
        nc.vector.tensor_copy(out=ident_r, in_=ident)
        ones = const.tile([1, 512], f32, tag="ones")
        nc.vector.memset(ones, 1.0)
        ones_r = const.tile([1, 512], f32r, tag="ones_r")
        nc.vector.tensor_copy(out=ones_r, in_=ones)
        zcol = const.tile([128, 4], f32, tag="zcol")
        nc.vector.memset(zcol, 0.0)
        eps = const.tile([128, 1], f32, tag="eps")
        nc.vector.memset(eps, 1e-5)

        cw = {}
        for nm, h in dp.items():
            t = const.tile(list(h.shape), h.dtype, tag=f"w_{nm}")
            nc.sync.dma_start(out=t, in_=h[:])
            cw[nm] = t

        mask_bt = const.tile([bpc, G], f32, tag="mask_bt")
        nc.sync.dma_start(out=mask_bt, in_=obs[:, :, 8])
        lv_bt = const.tile([bpc, LH], f32, tag="lv_bt")
        nc.sync.dma_start(out=lv_bt, in_=obs[:, IH:IH + LH, 8])

        compat_cols = const.tile([LH, max(bpc, 2)], f32, tag="compat_cols")

        def ecopy(eng, out, in_):
            if eng is nc.scalar:
                nc.scalar.copy(out=out, in_=in_)
            else:
                eng.tensor_copy(out=out, in_=in_)

        def mm(out, lhsT, rhs, start, stop):
            nc.tensor.matmul(out, lhsT, rhs, start=start, stop=stop)

        def zero_pad_cols(t3):
            for k in range(2):
                nc.gpsimd.tensor_copy(out=t3[:, k, G:G + 1], in_=zcol[:, 0:1])

        def transpose_to(dst_sb, src_sb, copy_eng):
            """dst[j, i] = src[i, j] via PE; src [p, f] -> dst [f, p]."""
            p, f = src_sb.shape
            idt = ident_r if src_sb.dtype == f32r else ident
            tp = ps.tile([128, 128], src_sb.dtype, tag="ps")
            nc.tensor.transpose(tp[:f, :p], src_sb, idt[:p, :p])
            ecopy(copy_eng, dst_sb, tp[:f, :p])

        # ================= per batch element =================
        for b in range(bpc):
            xT = st.tile([9, 304], f32, tag="xT")
            nc.sync.dma_start(out=xT[:, :G], in_=obs[b].rearrange("g f -> f g"))

            # ---- embedding MLPs -> h [301, 256] natural (3 g-chunk tiles) ----
            z1 = ps.tile([32, G], f32, tag="ps")
            mm(z1[:, :IH], cw["wi1"], xT[:6, :IH], True, False)
            mm(z1[:, :IH], cw["b1r_i"], ones[:, :IH], False, True)
            mm(z1[:, IH:IH + LH], cw["wl1"], xT[:8, IH:IH + LH], True, False)
            mm(z1[:, IH:IH + LH], cw["b1r_l"], ones[:, :LH], False, True)
            mm(z1[:, IH + LH:], cw["wn1"], xT[:6, IH + LH:G], True, False)
            mm(z1[:, IH + LH:], cw["b1r_n"], ones[:, :1], False, True)

            if dbg and b == 0:
                z1c = st.tile([32, G], f32, tag="z1c")
                nc.gpsimd.tensor_copy(out=z1c, in_=z1) if False else nc.vector.tensor_copy(out=z1c, in_=z1)
                nc.sync.dma_start(out=dbg_z1[:], in_=z1c)
            small1 = st.tile([32, G], f32, tag="small1")
            nc.scalar.activation(small1, z1, AF.Identity, scale=0.01)
            lr = st.tile([32, G], f32r, tag="lr")
            nc.vector.tensor_tensor(out=lr, in0=z1, in1=small1, op=ALU.max)

            if dbg and b == 0:
                nc.sync.dma_start(out=dbg_lr[:], in_=lr.bitcast(f32))
            h0 = st.tile([128, 3, E], f32, tag="h0")
            segs = [(0, 128, "i"), (128, IH, "i"), (IH, 256, "l"), (256, 300, "l"), (300, 301, "n")]
            for si, (c0, c1, ty) in enumerate(segs):
                ti, r0 = c0 // 128, c0 % 128
                rows = c1 - c0
                hp = ps.tile([128, E], f32, tag="ps", name=f"hp{si}")
                mm(hp[:rows, :], lr[:, c0:c1], cw[f"w2_{ty}"], True, False)
                mm(hp[:rows, :], ones_r[:, :rows], cw[f"b2r_{ty}"], False, True)
                eng = nc.scalar if si % 2 == 0 else nc.vector
                if r0 % 32 == 0:
                    ecopy(eng, h0[r0:r0 + rows, ti, :], hp[:rows, :])
                else:
                    stg = st.tile([128, E], f32, tag="stg", name=f"stg{si}")
                    ecopy(eng, stg[:rows, :], hp[:rows, :])
                    nc.sync.dma_start(out=h0[r0:r0 + rows, ti, :], in_=stg[:rows, :])

            # initial transpose h -> hT [e, g]
            hT = st.tile([128, 2, 304], f32r, tag="hT")
            for t in range(3):
                g0, g1 = GC[t]
                gs = g1 - g0
                for k in range(2):
                    eng = nc.scalar if (t + k) % 2 == 0 else nc.vector
                    transpose_to(hT[:, k, g0:g1], h0[:gs, t, k * 128:(k + 1) * 128], eng)

            if dbg and b == 0:
                nc.sync.dma_start(out=dbg_h0[:], in_=h0)
            h_nat = h0
            # ================= encoder layers =================
            for l in range(NL):
                # q^T, k^T [e, g]
                qT = st.tile([128, 2, 304], f32r, tag="qT")
                kT = st.tile([128, 2, 304], f32r, tag="kT")
                for m in range(2):
                    qp = ps.tile([128, VN], f32, tag="ps")
                    for k in range(2):
                        mm(qp, cw[f"wq{l}"][:, k, m * 128:(m + 1) * 128], hT[:, k, :VN], k == 0, k == 1)
                    nc.scalar.copy(qT[:, m, :VN], qp)
                    kp = ps.tile([128, VN], f32, tag="ps")
                    for k in range(2):
                        mm(kp, cw[f"wk{l}"][:, k, m * 128:(m + 1) * 128], hT[:, k, :VN], k == 0, k == 1)
                    nc.vector.tensor_copy(kT[:, m, :VN], kp)

                # v natural [g, e]
                v = st.tile([128, 3, E], f32r, tag="v")
                for t in range(3):
                    g0, g1 = GC[t]
                    gs = g1 - g0
                    vp = ps.tile([128, E], f32, tag="ps")
                    for k in range(2):
                        mm(vp[:gs, :], hT[:, k, g0:g1], cw[f"wv{l}"][:, k, :], k == 0, k == 1)
                    nc.scalar.copy(v[:gs, t, :], vp[:gs, :])

                # scores + softmax (unnormalized exp; 1/rowsum deferred to o)
                a = st.tile([128, 3, 304], f32r, tag="a")
                rinv = st.tile([128, 3], f32, tag="rinv")
                for t in range(3):
                    g0, g1 = GC[t]
                    gs = g1 - g0
                    sp = ps.tile([128, VN], f32, tag="ps")
                    for k in range(2):
                        mm(sp[:gs, :], qT[:, k, g0:g1], kT[:, k, :VN], k == 0, k == 1)
                    nmax = st.tile([128, 1], f32, tag="nmax")
                    nc.vector.reduce_max(nmax[:gs], sp[:gs, :G], axis=AX.X, negate=True)
                    rsum = st.tile([128, 1], f32, tag="rsum")
                    nc.scalar.activation(a[:gs, t, :G], sp[:gs, :G], AF.Exp,
                                         bias=nmax[:gs], accum_out=rsum[:gs])
                    nc.vector.reciprocal(rinv[:gs, t:t + 1], rsum[:gs])

                if dbg and b == 0 and l == 0:
                    nc.sync.dma_start(out=dbg_a[:], in_=a.bitcast(f32))
                    nc.sync.dma_start(out=dbg_rinv[:], in_=rinv)
                # a^T [gk, gq] via PE transposes
                aT = st.tile([128, 3, 304], f32r, tag="aT")
                for tq in range(3):
                    q0, q1 = GC2[tq]
                    qs = q1 - q0
                    for tk in range(3):
                        k0, k1 = GC2[tk]
                        ks = k1 - k0
                        eng = nc.scalar if (tq + tk) % 2 == 0 else nc.vector
                        tp = ps.tile([128, 128], f32r, tag="ps")
                        nc.tensor.transpose(tp[:ks, :qs], a[:qs, tq, k0:k1], ident_r[:qs, :qs])
                        ecopy(eng, aT[:ks, tk, q0:q1], tp[:ks, :qs])

                # av^T [e, gq] = sum_gk v^T a^T
                avT = st.tile([128, 2, 304], f32r, tag="avT")
                for m in range(2):
                    ap_ = ps.tile([128, VN], f32, tag="ps")
                    for tk in range(3):
                        k0, k1 = GC[tk]
                        ks = k1 - k0
                        mm(ap_, v[:ks, tk, m * 128:(m + 1) * 128], aT[:ks, tk, :VN], tk == 0, tk == 2)
                    eng = nc.scalar if m == 0 else nc.vector
                    ecopy(eng, avT[:, m, :VN], ap_)

                # o = av @ Wo (natural), scale rows by 1/rowsum, +h, LN -> h1
                h1 = st.tile([128, 3, E], f32, tag="h1")
                x1 = st.tile([128, 3, E], f32, tag="x1")
                for t in range(3):
                    g0, g1 = GC[t]
                    gs = g1 - g0
                    op_ = ps.tile([128, E], f32, tag="ps")
                    for k in range(2):
                        mm(op_[:gs, :], avT[:, k, g0:g1], cw[f"wo{l}"][:, k, :], k == 0, k == 1)
                    nc.scalar.activation(x1[:gs, t, :], op_[:gs, :], AF.Identity,
                                         scale=rinv[:gs, t:t + 1])
                    nc.vector.tensor_add(x1[:gs, t, :], x1[:gs, t, :], h_nat[:gs, t, :])
                    _layernorm(nc, st, eps, x1, h1, t, gs, f32, AF, ALU)

                # h1 -> h1T
                h1T = st.tile([128, 2, 304], f32r, tag="h1T")
                for t in range(3):
                    g0, g1 = GC[t]
                    gs = g1 - g0
                    for k in range(2):
                        eng = nc.scalar if (t + k) % 2 == 0 else nc.vector
                        transpose_to(h1T[:, k, g0:g1], h1[:gs, t, k * 128:(k + 1) * 128], eng)
                zero_pad_cols(h1T)

                # FF: f1^T = relu(wf1^T h1^T + bf1) [f, g]
                f1 = st.tile([128, 4, 304], f32r, tag="f1")
                for m in range(4):
                    fp = ps.tile([128, VN], f32, tag="ps")
                    for k in range(2):
                        mm(fp, cw[f"wf1{l}"][:, k, m * 128:(m + 1) * 128], h1T[:, k, :VN], k == 0, k == 1)
                    nc.scalar.activation(f1[:, m, :G], fp[:, :G], AF.Relu,
                                         bias=cw[f"bf1{l}"][:, m:m + 1])

                # f2 = f1 @ wf2 + bf2 (natural), +h1, LN -> h2
                h2 = st.tile([128, 3, E], f32, tag="h2")
                x2 = st.tile([128, 3, E], f32, tag="x2")
                for t in range(3):
                    g0, g1 = GC[t]
                    gs = g1 - g0
                    f2p = ps.tile([128, E], f32, tag="ps")
                    for m in range(4):
                        mm(f2p[:gs, :], f1[:, m, g0:g1], cw[f"wf2{l}"][:, m, :], m == 0, False)
                    mm(f2p[:gs, :], ones_r[:, :gs], cw[f"bf2{l}"], False, True)
                    nc.vector.tensor_add(x2[:gs, t, :], f2p[:gs, :], h1[:gs, t, :])
                    _layernorm(nc, st, eps, x2, h2, t, gs, f32, AF, ALU)

                if dbg and b == 0:
                    nc.sync.dma_start(out=(dbg_h1 if l == 0 else dbg_h2)[:], in_=h2)
                h_nat = h2
                if l < NL - 1:
                    hT = st.tile([128, 2, 304], f32r, tag="hT")
                    for t in range(3):
                        g0, g1 = GC[t]
                        gs = g1 - g0
                        for k in range(2):
                            eng = nc.scalar if (t + k) % 2 == 0 else nc.vector
                            transpose_to(hT[:, k, g0:g1], h2[:gs, t, k * 128:(k + 1) * 128], eng)
                    zero_pad_cols(hT)

            # ================= decoder =================
            mcol = st.tile([128, 3], f32, tag="mcol")
            trans = st.tile([128, 3, E], f32, tag="trans")
            for t in range(3):
                g0, g1 = GC[t]
                gs = g1 - g0
                nc.sync.dma_start(out=mcol[:gs, t:t + 1], in_=obs[b, g0:g1, 8:9])
                nc.vector.tensor_scalar_mul(trans[:gs, t, :], in0=h_nat[:gs, t, :],
                                            scalar1=mcol[:gs, t:t + 1])
            tT = st.tile([128, 2, 304], f32, tag="tT")
            for t in range(3):
                g0, g1 = GC[t]
                gs = g1 - g0
                for k in range(2):
                    eng = nc.scalar if (t + k) % 2 == 0 else nc.vector
                    transpose_to(tT[:, k, g0:g1], trans[:gs, t, k * 128:(k + 1) * 128], eng)

            ge = st.tile([128, 2], f32, tag="ge")
            for k in range(2):
                nc.vector.reduce_sum(ge[:, k:k + 1], tT[:, k, :G], axis=AX.X)

            c_sb = st.tile([128, 2], f32, tag="c_sb")
            for m in range(2):
                cp = ps.tile([128, 1], f32, tag="ps")
                for k in range(2):
                    mm(cp, cw["mt"][:, k, m * 128:(m + 1) * 128], ge[:, k:k + 1], k == 0, k == 1)
                nc.scalar.copy(c_sb[:, m:m + 1], cp)

            cp2 = ps.tile([LH, 1], f32, tag="ps", name="cp2")
            for k in range(2):
                mm(cp2, tT[:, k, IH:IH + LH], c_sb[:, k:k + 1], k == 0, k == 1)
            nc.scalar.copy(compat_cols[:, b:b + 1], cp2)

        # ================= batched tail =================
        vl = const.tile([bpc, 1], f32, tag="vl")
        nc.vector.reduce_sum(vl, mask_bt, axis=AX.X)
        ivl = const.tile([bpc, 1], f32, tag="ivl")
        nc.vector.reciprocal(ivl, vl)

        ctp = ps.tile([128, LH], f32, tag="ps", name="ctp")
        nc.tensor.transpose(ctp[:bpc, :LH], compat_cols[:, :bpc], ident[:LH, :LH])
        compat_sb = const.tile([bpc, LH], f32, tag="compat_sb")
        nc.vector.tensor_copy(compat_sb, ctp[:bpc, :LH])
        th = const.tile([bpc, LH], f32, tag="th")
        nc.scalar.activation(th, compat_sb, AF.Tanh, scale=ivl)
        ex = const.tile([bpc, LH], f32, tag="ex")
        es = const.tile([bpc, 1], f32, tag="es")
        nc.scalar.activation(ex, th, AF.Exp, scale=CLIP, accum_out=es)
        er = const.tile([bpc, 1], f32, tag="er")
        nc.vector.reciprocal(er, es)
        pm = const.tile([bpc, LH], f32, tag="pm")
        nc.vector.tensor_scalar_mul(pm, in0=ex, scalar1=er)
        nc.vector.tensor_tensor(out=pm, in0=pm, in1=lv_bt, op=mybir.AluOpType.mult)
        nc.vector.tensor_scalar_add(pm, in0=pm, scalar1=1e-20)
        rs2 = const.tile([bpc, 1], f32, tag="rs2")
        nc.vector.reduce_sum(rs2, pm, axis=AX.X)
        rr2 = const.tile([bpc, 1], f32, tag="rr2")
        nc.vector.reciprocal(rr2, rs2)
        ob = const.tile([bpc, LH], f32, tag="ob")
        nc.vector.tensor_scalar_mul(ob, in0=pm, scalar1=rr2)
        nc.sync.dma_start(out=out_d[:], in_=ob)

    nc.finalize()
    return nc


def _layernorm(nc, st, eps, x, h_out, t, gs, f32, AF, ALU):
    """LN over free dim (256) of x[:gs, t, :] -> h_out[:gs, t, :]."""
    import concourse.mybir as mybir
    st6 = st.tile([128, 6], f32, tag="st6")
    nc.vector.bn_stats(out=st6[:gs], in_=x[:gs, t, :])
    mv = st.tile([128, 2], f32, tag="mv")
    nc.vector.bn_aggr(out=mv[:gs], in_=st6[:gs])
    std = st.tile([128, 1], f32, tag="std")
    nc.scalar.activation(std[:gs], mv[:gs, 1:2], AF.Sqrt, bias=eps[:gs])
    rstd = st.tile([128, 1], f32, tag="rstd")
    nc.vector.reciprocal(rstd[:gs], std[:gs])
    nc.vector.tensor_scalar(
        out=h_out[:gs, t, :], in0=x[:gs, t, :],
        scalar1=mv[:gs, 0:1], scalar2=rstd[:gs],
        op0=ALU.subtract, op1=ALU.mult,
    )


# ----------------------------------------------------------------------------
# public entry point
# ----------------------------------------------------------------------------
def kernel(**inputs):
    observation = np.asarray(inputs["observation"], np.float32)
    w = _prep_weights(inputs)

    from concourse.bass_utils import run_bass_kernel_spmd

    nc = _build(BPC)
    in_maps = []
    for i in range(NCORES):
        m = {"obs": np.ascontiguousarray(observation[i * BPC:(i + 1) * BPC])}
        m.update(w)
        in_maps.append(m)
    res = run_bass_kernel_spmd(nc, in_maps, list(range(NCORES)))
    out = np.concatenate([res.results[i]["out"] for i in range(NCORES)], axis=0)
    return out.astype(np.float32)


# revision 22
# speedup vs baseline: 1.8798x; 1.8798x over previous
"""Trainium2 Bass kernel for nn_AttentionModel (graph attention encoder + decoder).

Contract: kernel(**inputs) takes FULL unsharded numpy inputs (as produced by
reference.setup_inputs()) and returns the FULL [256, 100] float32 output.
Internally shards the batch (256) across 8 NeuronCores (32 each, pure data
parallel; weights replicated) and runs a fused Bass/Tile kernel per core.

Self-contained: hardcodes all shapes; no sibling imports.
"""

import sys

for _p in ("/opt/trn_rl_repo", "/opt/pypackages"):
    if _p not in sys.path:
        sys.path.append(_p)

import numpy as np
from contextlib import ExitStack

# --- static architecture constants ---
B, IH, IL, LH, E, FFH, NL = 256, 200, 6, 100, 256, 512, 2
G = IH + LH + 1  # 301
CLIP = 10.0
SCALE = 1.0 / 16.0  # 1/sqrt(E)
NCORES = 8
BPC = B // NCORES  # 32 batch elements per core

# g chunks over 301 nodes, e chunks over 256 features, f chunks over 512
GC = [(0, 128), (128, 256), (256, 301)]
GC2 = [(0, 128), (128, 256), (256, 302)]  # even-padded for fp32r matmuls
VN = 302  # even-padded moving width over the node axis
ECN = 2  # e chunks of 128
FCN = 4  # ff chunks of 128


# ----------------------------------------------------------------------------
# host-side weight packing
# ----------------------------------------------------------------------------
def _tf32(x):
    """Round fp32 array to tfloat32 (10 mantissa bits), round-to-nearest-even."""
    u = np.ascontiguousarray(x, np.float32).view(np.uint32)
    u = (u + 0x0FFF + ((u >> 13) & 1)) & np.uint32(0xFFFFE000)
    return u.view(np.float32)


def _pack_rows(m, nchunk):
    """[nchunk*128, N] -> [128, nchunk, N] with [:, k, :] = m[128k:128(k+1), :]"""
    return np.ascontiguousarray(
        np.stack([m[i * 128:(i + 1) * 128] for i in range(nchunk)], axis=1)
    ).astype(np.float32)


def _prep_weights(inp):
    w = {}
    w["wi1"] = inp["wi1"].astype(np.float32)          # [6, 32]
    w["wl1"] = inp["wl1"].astype(np.float32)          # [8, 32]
    w["wn1"] = inp["wn1"].astype(np.float32)          # [6, 32]
    w["b1r_i"] = inp["bi1"].reshape(1, 32).astype(np.float32)
    w["b1r_l"] = inp["bl1"].reshape(1, 32).astype(np.float32)
    w["b1r_n"] = inp["bn1"].reshape(1, 32).astype(np.float32)
    w["w2_i"] = _tf32(inp["wi2"])         # [32, 256]
    w["w2_l"] = _tf32(inp["wl2"])
    w["w2_n"] = _tf32(inp["wn2"])
    w["b2r_i"] = _tf32(inp["bi2"].reshape(1, E))
    w["b2r_l"] = _tf32(inp["bl2"].reshape(1, E))
    w["b2r_n"] = _tf32(inp["bn2"].reshape(1, E))
    for l in range(NL):
        # fold the 1/sqrt(E) attention scale into Wq
        w[f"wq{l}"] = _tf32(_pack_rows(inp["enc_wq"][l] * SCALE, 2))   # [128,2,256] lhsT chunks
        w[f"wk{l}"] = _tf32(_pack_rows(inp["enc_wk"][l], 2))
        w[f"wv{l}"] = _tf32(_pack_rows(inp["enc_wv"][l], 2))           # rhs chunks
        w[f"wo{l}"] = _tf32(_pack_rows(inp["enc_wo"][l], 2))           # rhs chunks
        w[f"wf1{l}"] = _tf32(_pack_rows(inp["enc_wf1"][l], 2))         # [128,2,512] lhsT chunks
        w[f"bf1{l}"] = np.ascontiguousarray(
            inp["enc_bf1"][l].reshape(4, 128).T
        ).astype(np.float32)                                    # [128, 4]
        w[f"wf2{l}"] = _tf32(_pack_rows(inp["enc_wf2"][l], 4))         # [128,4,256] rhs chunks
        w[f"bf2{l}"] = _tf32(inp["enc_bf2"][l].reshape(1, E))
    # decoder fused matrix: compat = h_leaf . (M @ ge), M = Wpn_E @ Wfc.T
    # lhsT for c = M @ ge is MT = M.T = Wfc @ Wpn_E.T ; fold 1/sqrt(E) here
    MT = (inp["w_fc"] @ inp["w_pn"][:, :E].T) * SCALE
    w["mt"] = _pack_rows(MT, 2)                                 # [128,2,256]
    return w


# ----------------------------------------------------------------------------
# numpy mirror of the device computation (for algebra validation)
# ----------------------------------------------------------------------------
def _numpy_mirror(observation, w):
    obs = observation.astype(np.float32)
    nb = obs.shape[0]
    out = np.zeros((nb, LH), np.float32)

    def lrelu(x):
        return np.maximum(x, 0.01 * x)

    def ln(x):
        m = x.mean(-1, keepdims=True)
        v = x.var(-1, keepdims=True)
        return (x - m) / np.sqrt(v + 1e-5)

    for b in range(nb):
        xT = obs[b, :, :9].T  # [9, 301]
        h = np.zeros((G, E), np.float32)
        z_i = xT[:6, :IH].T @ w["wi1"] + w["b1r_i"]
        z_l = xT[:8, IH:IH + LH].T @ w["wl1"] + w["b1r_l"]
        z_n = xT[:6, IH + LH:].T @ w["wn1"] + w["b1r_n"]
        h[:IH] = lrelu(z_i) @ w["w2_i"] + w["b2r_i"]
        h[IH:IH + LH] = lrelu(z_l) @ w["w2_l"] + w["b2r_l"]
        h[IH + LH:] = lrelu(z_n) @ w["w2_n"] + w["b2r_n"]

        for l in range(NL):
            wq = np.concatenate([w[f"wq{l}"][:, 0], w[f"wq{l}"][:, 1]], 0)
            wk = np.concatenate([w[f"wk{l}"][:, 0], w[f"wk{l}"][:, 1]], 0)
            wv = np.concatenate([w[f"wv{l}"][:, 0], w[f"wv{l}"][:, 1]], 0)
            wo = np.concatenate([w[f"wo{l}"][:, 0], w[f"wo{l}"][:, 1]], 0)
            wf1 = np.concatenate([w[f"wf1{l}"][:, 0], w[f"wf1{l}"][:, 1]], 0)
            wf2 = np.concatenate([w[f"wf2{l}"][:, k] for k in range(4)], 0)
            bf1 = w[f"bf1{l}"].T.reshape(-1)
            q = h @ wq  # already scaled by 1/16
            k = h @ wk
            v = h @ wv
            s = q @ k.T
            mx = s.max(-1, keepdims=True)
            e = np.exp(s - mx)
            rs = e.sum(-1, keepdims=True)
            o = (e @ v) / rs @ wo
            h = ln(h + o)
            f = np.maximum(h @ wf1 + bf1, 0.0) @ wf2 + w[f"bf2{l}"]
            h = ln(h + f)

        mask = obs[b, :, 8]
        trans = h * mask[:, None]
        ge = trans.sum(0)  # unnormalized
        MT = np.concatenate([w["mt"][:, 0], w["mt"][:, 1]], 0)
        c = MT.T @ ge  # [256]
        compat = trans[IH:IH + LH] @ c  # [100]
        vlen = mask.sum()
        logits = np.tanh(compat / vlen) * CLIP
        ee = np.exp(logits)
        p = ee / ee.sum()
        lv = obs[b, IH:IH + LH, 8]
        masked = p * lv + 1e-20
        out[b] = masked / masked.sum()
    return out


# ----------------------------------------------------------------------------
# the Bass/Tile kernel
# ----------------------------------------------------------------------------
def _build(bpc, dbg=False, nbp=None):
    import concourse.bass as bass
    import concourse.mybir as mybir
    import concourse.tile as tile
    from concourse import bacc
    from concourse.masks import make_identity

    f32 = mybir.dt.float32
    f32r = mybir.dt.float32r
    AF = mybir.ActivationFunctionType
    ALU = mybir.AluOpType
    AX = mybir.AxisListType

    def r(ap):
        return ap.bitcast(f32r)

    nc = bacc.Bacc(None, target_bir_lowering=False)

    obs = nc.declare_dram_parameter("obs", [bpc, G, 9], f32, isOutput=False)
    dp = {}
    dp["wi1"] = nc.declare_dram_parameter("wi1", [6, 32], f32, isOutput=False)
    dp["wl1"] = nc.declare_dram_parameter("wl1", [8, 32], f32, isOutput=False)
    dp["wn1"] = nc.declare_dram_parameter("wn1", [6, 32], f32, isOutput=False)
    F32_WEIGHTS = {"wi1", "wl1", "wn1", "b1r_i", "b1r_l", "b1r_n", "bf10", "bf11"}
    for t in "iln":
        dp[f"b1r_{t}"] = nc.declare_dram_parameter(f"b1r_{t}", [1, 32], f32, isOutput=False)
        dp[f"w2_{t}"] = nc.declare_dram_parameter(f"w2_{t}", [32, E], f32r, isOutput=False)
        dp[f"b2r_{t}"] = nc.declare_dram_parameter(f"b2r_{t}", [1, E], f32r, isOutput=False)
    for l in range(NL):
        for nm, shp in (
            (f"wq{l}", [128, 2, E]), (f"wk{l}", [128, 2, E]),
            (f"wv{l}", [128, 2, E]), (f"wo{l}", [128, 2, E]),
            (f"wf1{l}", [128, 2, FFH]), (f"bf1{l}", [128, 4]),
            (f"wf2{l}", [128, 4, E]), (f"bf2{l}", [1, E]),
        ):
            wdt = f32 if nm in F32_WEIGHTS else f32r
            dp[nm] = nc.declare_dram_parameter(nm, shp, wdt, isOutput=False)
    dp["mt"] = nc.declare_dram_parameter("mt", [128, 2, E], f32, isOutput=False)
    out_d = nc.declare_dram_parameter("out", [bpc, LH], f32, isOutput=True)
    if dbg:
        dbg_h0 = nc.declare_dram_parameter("dbg_h0", [128, 3, E], f32, isOutput=True)
        dbg_a = nc.declare_dram_parameter("dbg_a", [128, 3, 304], f32, isOutput=True)
        dbg_h1 = nc.declare_dram_parameter("dbg_h1", [128, 3, E], f32, isOutput=True)
        dbg_h2 = nc.declare_dram_parameter("dbg_h2", [128, 3, E], f32, isOutput=True)
        dbg_rinv = nc.declare_dram_parameter("dbg_rinv", [128, 3], f32, isOutput=True)
        dbg_z1 = nc.declare_dram_parameter("dbg_z1", [32, G], f32, isOutput=True)
        dbg_lr = nc.declare_dram_parameter("dbg_lr", [32, G], f32, isOutput=True)

    with tile.TileContext(nc) as tc, ExitStack() as ctx:
        const = ctx.enter_context(tc.tile_pool(name="const", bufs=1))
        st = ctx.enter_context(tc.tile_pool(name="st", bufs=2))
        sm = ctx.enter_context(tc.tile_pool(name="sm", bufs=3))
        ps = ctx.enter_context(tc.tile_pool(name="ps", bufs=6, space="PSUM"))

        # ---- constants / weights into SBUF ----
        ident = const.tile([128, 128], f32, tag="ident")
        make_identity(nc, ident)
        ident_r = const.tile([128, 128], f32r, tag="ident_r")
        nc.vector.tensor_copy(out=ident_r, in_=ident)
        ones = const.tile([1, 512], f32, tag="ones")
        nc.vector.memset(ones, 1.0)
        ones_r = const.tile([1, 512], f32r, tag="ones_r")
        nc.vector.tensor_copy(out=ones_r, in_=ones)
        zcol = const.tile([128, 4], f32, tag="zcol")
        nc.vector.memset(zcol, 0.0)
        eps = const.tile([128, 1], f32, tag="eps")
        nc.vector.memset(eps, 1e-5)

        cw = {}
        for nm, h in dp.items():
            t = const.tile(list(h.shape), h.dtype, tag=f"w_{nm}")
            nc.sync.dma_start(out=t, in_=h[:])
            cw[nm] = t

        mask_bt = const.tile([bpc, G], f32, tag="mask_bt")
        nc.sync.dma_start(out=mask_bt, in_=obs[:, :, 8])
        lv_bt = const.tile([bpc, LH], f32, tag="lv_bt")
        nc.sync.dma_start(out=lv_bt, in_=obs[:, IH:IH + LH, 8])

        compat_cols = const.tile([LH, max(bpc, 2)], f32, tag="compat_cols")

        def ecopy(eng, out, in_):
            if eng is nc.scalar:
                nc.scalar.copy(out=out, in_=in_)
            else:
                eng.tensor_copy(out=out, in_=in_)

        def mm(out, lhsT, rhs, start, stop):
            nc.tensor.matmul(out, lhsT, rhs, start=start, stop=stop)

        def zero_pad_cols(t3):
            for k in range(2):
                nc.gpsimd.tensor_copy(out=t3[:, k, G:G + 1], in_=zcol[:, 0:1])

        def transpose_to(dst_sb, src_sb, copy_eng):
            """dst[j, i] = src[i, j] via PE; src [p, f] -> dst [f, p]."""
            p, f = src_sb.shape
            idt = ident_r if src_sb.dtype == f32r else ident
            tp = ps.tile([128, 128], src_sb.dtype, tag="ps")
            nc.tensor.transpose(tp[:f, :p], src_sb, idt[:p, :p])
            ecopy(copy_eng, dst_sb, tp[:f, :p])

        # ================= per batch element =================
        for b in range(nbp if nbp is not None else bpc):
            xT = st.tile([9, 304], f32, tag="xT")
            nc.sync.dma_start(out=xT[:, :G], in_=obs[b].rearrange("g f -> f g"))

            # ---- embedding MLPs -> h [301, 256] natural (3 g-chunk tiles) ----
            z1 = ps.tile([32, G], f32, tag="ps")
            mm(z1[:, :IH], cw["wi1"], xT[:6, :IH], True, False)
            mm(z1[:, :IH], cw["b1r_i"], ones[:, :IH], False, True)
            mm(z1[:, IH:IH + LH], cw["wl1"], xT[:8, IH:IH + LH], True, False)
            mm(z1[:, IH:IH + LH], cw["b1r_l"], ones[:, :LH], False, True)
            mm(z1[:, IH + LH:], cw["wn1"], xT[:6, IH + LH:G], True, False)
            mm(z1[:, IH + LH:], cw["b1r_n"], ones[:, :1], False, True)

            if dbg and b == 0:
                z1c = st.tile([32, G], f32, tag="z1c")
                nc.gpsimd.tensor_copy(out=z1c, in_=z1) if False else nc.vector.tensor_copy(out=z1c, in_=z1)
                nc.sync.dma_start(out=dbg_z1[:], in_=z1c)
            small1 = st.tile([32, G], f32, tag="small1")
            nc.scalar.activation(small1, z1, AF.Identity, scale=0.01)
            lr = st.tile([32, G], f32r, tag="lr")
            nc.vector.tensor_tensor(out=lr, in0=z1, in1=small1, op=ALU.max)

            if dbg and b == 0:
                nc.sync.dma_start(out=dbg_lr[:], in_=lr.bitcast(f32))
            h0 = st.tile([128, 3, E], f32, tag="h0")
            segs = [(0, 128, "i"), (128, IH, "i"), (IH, 256, "l"), (256, 300, "l"), (300, 301, "n")]
            for si, (c0, c1, ty) in enumerate(segs):
                ti, r0 = c0 // 128, c0 % 128
                rows = c1 - c0
                hp = ps.tile([128, E], f32, tag="ps", name=f"hp{si}")
                mm(hp[:rows, :], lr[:, c0:c1], cw[f"w2_{ty}"], True, False)
                mm(hp[:rows, :], ones_r[:, :rows], cw[f"b2r_{ty}"], False, True)
                eng = nc.scalar if si % 2 == 0 else nc.vector
                if r0 % 32 == 0:
                    ecopy(eng, h0[r0:r0 + rows, ti, :], hp[:rows, :])
                else:
                    stg = st.tile([128, E], f32, tag="stg", name=f"stg{si}")
                    ecopy(eng, stg[:rows, :], hp[:rows, :])
                    nc.sync.dma_start(out=h0[r0:r0 + rows, ti, :], in_=stg[:rows, :])

            # initial transpose h -> hT [e, g]
            hT = st.tile([128, 2, 304], f32r, tag="hT")
            for t in range(3):
                g0, g1 = GC[t]
                gs = g1 - g0
                for k in range(2):
                    eng = nc.scalar if (t + k) % 2 == 0 else nc.vector
                    transpose_to(hT[:, k, g0:g1], h0[:gs, t, k * 128:(k + 1) * 128], eng)

            if dbg and b == 0:
                nc.sync.dma_start(out=dbg_h0[:], in_=h0)
            h_nat = h0
            # ================= encoder layers =================
            for l in range(NL):
                # q^T, k^T [e, g]
                qT = st.tile([128, 2, 304], f32r, tag="qT")
                kT = st.tile([128, 2, 304], f32r, tag="kT")
                for m in range(2):
                    qp = ps.tile([128, VN], f32, tag="ps")
                    for k in range(2):
                        mm(qp, cw[f"wq{l}"][:, k, m * 128:(m + 1) * 128], hT[:, k, :VN], k == 0, k == 1)
                    nc.scalar.copy(qT[:, m, :VN], qp)
                    kp = ps.tile([128, VN], f32, tag="ps")
                    for k in range(2):
                        mm(kp, cw[f"wk{l}"][:, k, m * 128:(m + 1) * 128], hT[:, k, :VN], k == 0, k == 1)
                    nc.vector.tensor_copy(kT[:, m, :VN], kp)

                # v natural [g, e]
                v = st.tile([128, 3, E], f32r, tag="v")
                for t in range(3):
                    g0, g1 = GC[t]
                    gs = g1 - g0
                    vp = ps.tile([128, E], f32, tag="ps")
                    for k in range(2):
                        mm(vp[:gs, :], hT[:, k, g0:g1], cw[f"wv{l}"][:, k, :], k == 0, k == 1)
                    nc.scalar.copy(v[:gs, t, :], vp[:gs, :])

                # scores + softmax (unnormalized exp; 1/rowsum deferred to o)
                a = st.tile([128, 3, 304], f32r, tag="a")
                rinv = st.tile([128, 3], f32, tag="rinv")
                for t in range(3):
                    g0, g1 = GC[t]
                    gs = g1 - g0
                    sp = ps.tile([128, VN], f32, tag="ps")
                    for k in range(2):
                        mm(sp[:gs, :], qT[:, k, g0:g1], kT[:, k, :VN], k == 0, k == 1)
                    nmax = st.tile([128, 1], f32, tag="nmax")
                    nc.vector.reduce_max(nmax[:gs], sp[:gs, :G], axis=AX.X, negate=True)
                    rsum = st.tile([128, 1], f32, tag="rsum")
                    nc.scalar.activation(a[:gs, t, :G], sp[:gs, :G], AF.Exp,
                                         bias=nmax[:gs], accum_out=rsum[:gs])
                    nc.vector.reciprocal(rinv[:gs, t:t + 1], rsum[:gs])

                if dbg and b == 0 and l == 0:
                    nc.sync.dma_start(out=dbg_a[:], in_=a.bitcast(f32))
                    nc.sync.dma_start(out=dbg_rinv[:], in_=rinv)
                # a^T [gk, gq] via PE transposes
                aT = st.tile([128, 3, 304], f32r, tag="aT")
                for tq in range(3):
                    q0, q1 = GC2[tq]
                    qs = q1 - q0
                    for tk in range(3):
                        k0, k1 = GC2[tk]
                        ks = k1 - k0
                        eng = nc.scalar if (tq + tk) % 2 == 0 else nc.vector
                        tp = ps.tile([128, 128], f32r, tag="ps")
                        nc.tensor.transpose(tp[:ks, :qs], a[:qs, tq, k0:k1], ident_r[:qs, :qs])
                        ecopy(eng, aT[:ks, tk, q0:q1], tp[:ks, :qs])

                # av^T [e, gq] = sum_gk v^T a^T
                avT = st.tile([128, 2, 304], f32r, tag="avT")
                for m in range(2):
                    ap_ = ps.tile([128, VN], f32, tag="ps")
                    for tk in range(3):
                        k0, k1 = GC[tk]
                        ks = k1 - k0
                        mm(ap_, v[:ks, tk, m * 128:(m + 1) * 128], aT[:ks, tk, :VN], tk == 0, tk == 2)
                    eng = nc.scalar if m == 0 else nc.vector
                    ecopy(eng, avT[:, m, :VN], ap_)

                # o = av @ Wo (natural), scale rows by 1/rowsum, +h, LN -> h1
                h1 = st.tile([128, 3, E], f32, tag="h1")
                x1 = st.tile([128, 3, E], f32, tag="x1")
                for t in range(3):
                    g0, g1 = GC[t]
                    gs = g1 - g0
                    op_ = ps.tile([128, E], f32, tag="ps")
                    for k in range(2):
                        mm(op_[:gs, :], avT[:, k, g0:g1], cw[f"wo{l}"][:, k, :], k == 0, k == 1)
                    nc.scalar.activation(x1[:gs, t, :], op_[:gs, :], AF.Identity,
                                         scale=rinv[:gs, t:t + 1])
                    nc.vector.tensor_add(x1[:gs, t, :], x1[:gs, t, :], h_nat[:gs, t, :])
                    _layernorm(nc, st, eps, x1, h1, t, gs, f32, AF, ALU)

                # h1 -> h1T
                h1T = st.tile([128, 2, 304], f32r, tag="h1T")
                for t in range(3):
                    g0, g1 = GC[t]
                    gs = g1 - g0
                    for k in range(2):
                        eng = nc.scalar if (t + k) % 2 == 0 else nc.vector
                        transpose_to(h1T[:, k, g0:g1], h1[:gs, t, k * 128:(k + 1) * 128], eng)
                zero_pad_cols(h1T)

                # FF: f1^T = relu(wf1^T h1^T + bf1) [f, g]
                f1 = st.tile([128, 4, 304], f32r, tag="f1")
                for m in range(4):
                    fp = ps.tile([128, VN], f32, tag="ps")
                    for k in range(2):
                        mm(fp, cw[f"wf1{l}"][:, k, m * 128:(m + 1) * 128], h1T[:, k, :VN], k == 0, k == 1)
                    nc.scalar.activation(f1[:, m, :G], fp[:, :G], AF.Relu,
                                         bias=cw[f"bf1{l}"][:, m:m + 1])

                # f2 = f1 @ wf2 + bf2 (natural), +h1, LN -> h2
                h2 = st.tile([128, 3, E], f32, tag="h2")
                x2 = st.tile([128, 3, E], f32, tag="x2")
                for t in range(3):
                    g0, g1 = GC[t]
                    gs = g1 - g0
                    f2p = ps.tile([128, E], f32, tag="ps")
                    for m in range(4):
                        mm(f2p[:gs, :], f1[:, m, g0:g1], cw[f"wf2{l}"][:, m, :], m == 0, False)
                    mm(f2p[:gs, :], ones_r[:, :gs], cw[f"bf2{l}"], False, True)
                    nc.vector.tensor_add(x2[:gs, t, :], f2p[:gs, :], h1[:gs, t, :])
                    _layernorm(nc, st, eps, x2, h2, t, gs, f32, AF, ALU)

                if dbg and b == 0:
                    nc.sync.dma_start(out=(dbg_h1 if l == 0 else dbg_h2)[:], in_=h2)
                h_nat = h2
                if l < NL - 1:
                    hT = st.tile([128, 2, 304], f32r, tag="hT")
                    for t in range(3):
                        g0, g1 = GC[t]
                        gs = g1 - g0
                        for k in range(2):
                            eng = nc.scalar if (t + k) % 2 == 0 else nc.vector
                            transpose_to(hT[:, k, g0:g1], h2[:gs, t, k * 128:(k + 1) * 128], eng)
                    zero_pad_cols(hT)

            # ================= decoder =================
            mcol = st.tile([128, 3], f32, tag="mcol")
            trans = st.tile([128, 3, E], f32, tag="trans")
            for t in range(3):
                g0, g1 = GC[t]
                gs = g1 - g0
                nc.sync.dma_start(out=mcol[:gs, t:t + 1], in_=obs[b, g0:g1, 8:9])
                nc.vector.tensor_scalar_mul(trans[:gs, t, :], in0=h_nat[:gs, t, :],
                                            scalar1=mcol[:gs, t:t + 1])
            tT = st.tile([128, 2, 304], f32, tag="tT")
            for t in range(3):
                g0, g1 = GC[t]
                gs = g1 - g0
                for k in range(2):
                    eng = nc.scalar if (t + k) % 2 == 0 else nc.vector
                    transpose_to(tT[:, k, g0:g1], trans[:gs, t, k * 128:(k + 1) * 128], eng)

            ge = st.tile([128, 2], f32, tag="ge")
            for k in range(2):
                nc.vector.reduce_sum(ge[:, k:k + 1], tT[:, k, :G], axis=AX.X)

            c_sb = st.tile([128, 2], f32, tag="c_sb")
            for m in range(2):
                cp = ps.tile([128, 1], f32, tag="ps")
                for k in range(2):
                    mm(cp, cw["mt"][:, k, m * 128:(m + 1) * 128], ge[:, k:k + 1], k == 0, k == 1)
                nc.scalar.copy(c_sb[:, m:m + 1], cp)

            cp2 = ps.tile([LH, 1], f32, tag="ps", name="cp2")
            for k in range(2):
                mm(cp2, tT[:, k, IH:IH + LH], c_sb[:, k:k + 1], k == 0, k == 1)
            nc.scalar.copy(compat_cols[:, b:b + 1], cp2)

        # ================= batched tail =================
        vl = const.tile([bpc, 1], f32, tag="vl")
        nc.vector.reduce_sum(vl, mask_bt, axis=AX.X)
        ivl = const.tile([bpc, 1], f32, tag="ivl")
        nc.vector.reciprocal(ivl, vl)

        ctp = ps.tile([128, LH], f32, tag="ps", name="ctp")
        nc.tensor.transpose(ctp[:bpc, :LH], compat_cols[:, :bpc], ident[:LH, :LH])
        compat_sb = const.tile([bpc, LH], f32, tag="compat_sb")
        nc.vector.tensor_copy(compat_sb, ctp[:bpc, :LH])
        th = const.tile([bpc, LH], f32, tag="th")
        nc.scalar.activation(th, compat_sb, AF.Tanh, scale=ivl)
        ex = const.tile([bpc, LH], f32, tag="ex")
        es = const.tile([bpc, 1], f32, tag="es")
        nc.scalar.activation(ex, th, AF.Exp, scale=CLIP, accum_out=es)
        er = const.tile([bpc, 1], f32, tag="er")
        nc.vector.reciprocal(er, es)
        pm = const.tile([bpc, LH], f32, tag="pm")
        nc.vector.tensor_scalar_mul(pm, in0=ex, scalar1=er)
        nc.vector.tensor_tensor(out=pm, in0=pm, in1=lv_bt, op=mybir.AluOpType.mult)
        nc.vector.tensor_scalar_add(pm, in0=pm, scalar1=1e-20)
        rs2 = const.tile([bpc, 1], f32, tag="rs2")
        nc.vector.reduce_sum(rs2, pm, axis=AX.X)
        rr2 = const.tile([bpc, 1], f32, tag="rr2")
        nc.vector.reciprocal(rr2, rs2)
        ob = const.tile([bpc, LH], f32, tag="ob")
        nc.vector.tensor_scalar_mul(ob, in0=pm, scalar1=rr2)
        nc.sync.dma_start(out=out_d[:], in_=ob)

    nc.finalize()
    return nc


def _layernorm(nc, st, eps, x, h_out, t, gs, f32, AF, ALU):
    """LN over free dim (256) of x[:gs, t, :] -> h_out[:gs, t, :]."""
    import concourse.mybir as mybir
    st6 = st.tile([128, 6], f32, tag="st6")
    nc.vector.bn_stats(out=st6[:gs], in_=x[:gs, t, :])
    mv = st.tile([128, 2], f32, tag="mv")
    nc.vector.bn_aggr(out=mv[:gs], in_=st6[:gs])
    std = st.tile([128, 1], f32, tag="std")
    nc.scalar.activation(std[:gs], mv[:gs, 1:2], AF.Sqrt, bias=eps[:gs])
    rstd = st.tile([128, 1], f32, tag="rstd")
    nc.vector.reciprocal(rstd[:gs], std[:gs])
    nc.vector.tensor_scalar(
        out=h_out[:gs, t, :], in0=x[:gs, t, :],
        scalar1=mv[:gs, 0:1], scalar2=rstd[:gs],
        op0=ALU.subtract, op1=ALU.mult,
    )


# ----------------------------------------------------------------------------
# public entry point
# ----------------------------------------------------------------------------
def kernel(**inputs):
    observation = np.asarray(inputs["observation"], np.float32)
    w = _prep_weights(inputs)

    from concourse.bass_utils import run_bass_kernel_spmd

    nc = _build(BPC)
    in_maps = []
    for i in range(NCORES):
        m = {"obs": np.ascontiguousarray(observation[i * BPC:(i + 1) * BPC])}
        m.update(w)
        in_maps.append(m)
    res = run_bass_kernel_spmd(nc, in_maps, list(range(NCORES)))
    out = np.concatenate([res.results[i]["out"] for i in range(NCORES)], axis=0)
    return out.astype(np.float32)


# revision 25
# speedup vs baseline: 6.2101x; 3.3035x over previous
"""Trainium2 Bass kernel for nn_AttentionModel (graph attention encoder + decoder).

Contract: kernel(**inputs) takes FULL unsharded numpy inputs (as produced by
reference.setup_inputs()) and returns the FULL [256, 100] float32 output.
Internally shards the batch (256) across 8 NeuronCores (32 each, pure data
parallel; weights replicated) and runs a fused Bass/Tile kernel per core.

Self-contained: hardcodes all shapes; no sibling imports.
"""

import sys

for _p in ("/opt/trn_rl_repo", "/opt/pypackages"):
    if _p not in sys.path:
        sys.path.append(_p)

import numpy as np
from contextlib import ExitStack

# --- static architecture constants ---
B, IH, IL, LH, E, FFH, NL = 256, 200, 6, 100, 256, 512, 2
G = IH + LH + 1  # 301
CLIP = 10.0
SCALE = 1.0 / 16.0  # 1/sqrt(E)
NCORES = 8
BPC = B // NCORES  # 32 batch elements per core

# g chunks over 301 nodes, e chunks over 256 features, f chunks over 512
GC = [(0, 128), (128, 256), (256, 301)]
GC2 = [(0, 128), (128, 256), (256, 302)]  # even-padded for fp32r matmuls
VN = 302  # even-padded moving width over the node axis
ECN = 2  # e chunks of 128
FCN = 4  # ff chunks of 128


# ----------------------------------------------------------------------------
# host-side weight packing
# ----------------------------------------------------------------------------
def _tf32(x):
    """Round fp32 array to tfloat32 (10 mantissa bits), round-to-nearest-even."""
    u = np.ascontiguousarray(x, np.float32).view(np.uint32)
    u = (u + 0x0FFF + ((u >> 13) & 1)) & np.uint32(0xFFFFE000)
    return u.view(np.float32)


def _pack_rows(m, nchunk):
    """[nchunk*128, N] -> [128, nchunk, N] with [:, k, :] = m[128k:128(k+1), :]"""
    return np.ascontiguousarray(
        np.stack([m[i * 128:(i + 1) * 128] for i in range(nchunk)], axis=1)
    ).astype(np.float32)


def _prep_weights(inp):
    w = {}
    w["wi1"] = inp["wi1"].astype(np.float32)          # [6, 32]
    w["wl1"] = inp["wl1"].astype(np.float32)          # [8, 32]
    w["wn1"] = inp["wn1"].astype(np.float32)          # [6, 32]
    w["b1r_i"] = inp["bi1"].reshape(1, 32).astype(np.float32)
    w["b1r_l"] = inp["bl1"].reshape(1, 32).astype(np.float32)
    w["b1r_n"] = inp["bn1"].reshape(1, 32).astype(np.float32)
    w["w2_i"] = _tf32(inp["wi2"])         # [32, 256]
    w["w2_l"] = _tf32(inp["wl2"])
    w["w2_n"] = _tf32(inp["wn2"])
    w["b2r_i"] = _tf32(inp["bi2"].reshape(1, E))
    w["b2r_l"] = _tf32(inp["bl2"].reshape(1, E))
    w["b2r_n"] = _tf32(inp["bn2"].reshape(1, E))
    for l in range(NL):
        # fold the 1/sqrt(E) attention scale into Wq
        w[f"wq{l}"] = _tf32(_pack_rows(inp["enc_wq"][l] * SCALE, 2))   # [128,2,256] lhsT chunks
        w[f"wk{l}"] = _tf32(_pack_rows(inp["enc_wk"][l], 2))
        w[f"wv{l}"] = _tf32(_pack_rows(inp["enc_wv"][l], 2))           # rhs chunks
        w[f"wo{l}"] = _tf32(_pack_rows(inp["enc_wo"][l], 2))           # rhs chunks
        w[f"wf1{l}"] = _tf32(_pack_rows(inp["enc_wf1"][l], 2))         # [128,2,512] lhsT chunks
        w[f"bf1{l}"] = np.ascontiguousarray(
            inp["enc_bf1"][l].reshape(4, 128).T
        ).astype(np.float32)                                    # [128, 4]
        w[f"wf2{l}"] = _tf32(_pack_rows(inp["enc_wf2"][l], 4))         # [128,4,256] rhs chunks
        w[f"bf2{l}"] = _tf32(inp["enc_bf2"][l].reshape(1, E))
    # decoder fused matrix: compat = h_leaf . (M @ ge), M = Wpn_E @ Wfc.T
    # lhsT for c = M @ ge is MT = M.T = Wfc @ Wpn_E.T ; fold 1/sqrt(E) here
    MT = (inp["w_fc"] @ inp["w_pn"][:, :E].T) * SCALE
    w["mt"] = _pack_rows(MT, 2)                                 # [128,2,256]
    return w


# ----------------------------------------------------------------------------
# numpy mirror of the device computation (for algebra validation)
# ----------------------------------------------------------------------------
def _numpy_mirror(observation, w):
    obs = observation.astype(np.float32)
    nb = obs.shape[0]
    out = np.zeros((nb, LH), np.float32)

    def lrelu(x):
        return np.maximum(x, 0.01 * x)

    def ln(x):
        m = x.mean(-1, keepdims=True)
        v = x.var(-1, keepdims=True)
        return (x - m) / np.sqrt(v + 1e-5)

    for b in range(nb):
        xT = obs[b, :, :9].T  # [9, 301]
        h = np.zeros((G, E), np.float32)
        z_i = xT[:6, :IH].T @ w["wi1"] + w["b1r_i"]
        z_l = xT[:8, IH:IH + LH].T @ w["wl1"] + w["b1r_l"]
        z_n = xT[:6, IH + LH:].T @ w["wn1"] + w["b1r_n"]
        h[:IH] = lrelu(z_i) @ w["w2_i"] + w["b2r_i"]
        h[IH:IH + LH] = lrelu(z_l) @ w["w2_l"] + w["b2r_l"]
        h[IH + LH:] = lrelu(z_n) @ w["w2_n"] + w["b2r_n"]

        for l in range(NL):
            wq = np.concatenate([w[f"wq{l}"][:, 0], w[f"wq{l}"][:, 1]], 0)
            wk = np.concatenate([w[f"wk{l}"][:, 0], w[f"wk{l}"][:, 1]], 0)
            wv = np.concatenate([w[f"wv{l}"][:, 0], w[f"wv{l}"][:, 1]], 0)
            wo = np.concatenate([w[f"wo{l}"][:, 0], w[f"wo{l}"][:, 1]], 0)
            wf1 = np.concatenate([w[f"wf1{l}"][:, 0], w[f"wf1{l}"][:, 1]], 0)
            wf2 = np.concatenate([w[f"wf2{l}"][:, k] for k in range(4)], 0)
            bf1 = w[f"bf1{l}"].T.reshape(-1)
            q = h @ wq  # already scaled by 1/16
            k = h @ wk
            v = h @ wv
            s = q @ k.T
            mx = s.max(-1, keepdims=True)
            e = np.exp(s - mx)
            rs = e.sum(-1, keepdims=True)
            o = (e @ v) / rs @ wo
            h = ln(h + o)
            f = np.maximum(h @ wf1 + bf1, 0.0) @ wf2 + w[f"bf2{l}"]
            h = ln(h + f)

        mask = obs[b, :, 8]
        trans = h * mask[:, None]
        ge = trans.sum(0)  # unnormalized
        MT = np.concatenate([w["mt"][:, 0], w["mt"][:, 1]], 0)
        c = MT.T @ ge  # [256]
        compat = trans[IH:IH + LH] @ c  # [100]
        vlen = mask.sum()
        logits = np.tanh(compat / vlen) * CLIP
        ee = np.exp(logits)
        p = ee / ee.sum()
        lv = obs[b, IH:IH + LH, 8]
        masked = p * lv + 1e-20
        out[b] = masked / masked.sum()
    return out


# ----------------------------------------------------------------------------
# the Bass/Tile kernel
# ----------------------------------------------------------------------------
def _build(bpc, dbg=False, nbp=None):
    import concourse.bass as bass
    import concourse.mybir as mybir
    import concourse.tile as tile
    from concourse import bacc
    from concourse.masks import make_identity

    f32 = mybir.dt.float32
    f32r = mybir.dt.float32r
    AF = mybir.ActivationFunctionType
    ALU = mybir.AluOpType
    AX = mybir.AxisListType

    def r(ap):
        return ap.bitcast(f32r)

    nc = bacc.Bacc(None, target_bir_lowering=False)

    obs = nc.declare_dram_parameter("obs", [bpc, G, 9], f32, isOutput=False)
    dp = {}
    dp["wi1"] = nc.declare_dram_parameter("wi1", [6, 32], f32, isOutput=False)
    dp["wl1"] = nc.declare_dram_parameter("wl1", [8, 32], f32, isOutput=False)
    dp["wn1"] = nc.declare_dram_parameter("wn1", [6, 32], f32, isOutput=False)
    F32_WEIGHTS = {"wi1", "wl1", "wn1", "b1r_i", "b1r_l", "b1r_n", "bf10", "bf11"}
    for t in "iln":
        dp[f"b1r_{t}"] = nc.declare_dram_parameter(f"b1r_{t}", [1, 32], f32, isOutput=False)
        dp[f"w2_{t}"] = nc.declare_dram_parameter(f"w2_{t}", [32, E], f32r, isOutput=False)
        dp[f"b2r_{t}"] = nc.declare_dram_parameter(f"b2r_{t}", [1, E], f32r, isOutput=False)
    for l in range(NL):
        for nm, shp in (
            (f"wq{l}", [128, 2, E]), (f"wk{l}", [128, 2, E]),
            (f"wv{l}", [128, 2, E]), (f"wo{l}", [128, 2, E]),
            (f"wf1{l}", [128, 2, FFH]), (f"bf1{l}", [128, 4]),
            (f"wf2{l}", [128, 4, E]), (f"bf2{l}", [1, E]),
        ):
            wdt = f32 if nm in F32_WEIGHTS else f32r
            dp[nm] = nc.declare_dram_parameter(nm, shp, wdt, isOutput=False)
    dp["mt"] = nc.declare_dram_parameter("mt", [128, 2, E], f32, isOutput=False)
    out_d = nc.declare_dram_parameter("out", [bpc, LH], f32, isOutput=True)
    if dbg:
        dbg_h0 = nc.declare_dram_parameter("dbg_h0", [128, 3, E], f32, isOutput=True)
        dbg_a = nc.declare_dram_parameter("dbg_a", [128, 3, 304], f32, isOutput=True)
        dbg_h1 = nc.declare_dram_parameter("dbg_h1", [128, 3, E], f32, isOutput=True)
        dbg_h2 = nc.declare_dram_parameter("dbg_h2", [128, 3, E], f32, isOutput=True)
        dbg_rinv = nc.declare_dram_parameter("dbg_rinv", [128, 3], f32, isOutput=True)
        dbg_z1 = nc.declare_dram_parameter("dbg_z1", [32, G], f32, isOutput=True)
        dbg_lr = nc.declare_dram_parameter("dbg_lr", [32, G], f32, isOutput=True)

    with tile.TileContext(nc) as tc, ExitStack() as ctx:
        const = ctx.enter_context(tc.tile_pool(name="const", bufs=1))
        st = ctx.enter_context(tc.tile_pool(name="st", bufs=3))
        sm = ctx.enter_context(tc.tile_pool(name="sm", bufs=3))
        ps = ctx.enter_context(tc.tile_pool(name="ps", bufs=7, space="PSUM"))

        # ---- constants / weights into SBUF ----
        ident = const.tile([128, 128], f32, tag="ident")
        make_identity(nc, ident)
        ident_r = const.tile([128, 128], f32r, tag="ident_r")
        nc.vector.tensor_copy(out=ident_r, in_=ident)
        ones = const.tile([1, 512], f32, tag="ones")
        nc.vector.memset(ones, 1.0)
        ones_r = const.tile([1, 512], f32r, tag="ones_r")
        nc.vector.tensor_copy(out=ones_r, in_=ones)
        zcol = const.tile([128, 4], f32, tag="zcol")
        nc.vector.memset(zcol, 0.0)
        eps = const.tile([128, 1], f32, tag="eps")
        nc.vector.memset(eps, 1e-5)

        cw = {}
        for nm, h in dp.items():
            t = const.tile(list(h.shape), h.dtype, tag=f"w_{nm}")
            nc.sync.dma_start(out=t, in_=h[:])
            cw[nm] = t

        mask_bt = const.tile([bpc, G], f32, tag="mask_bt")
        nc.sync.dma_start(out=mask_bt, in_=obs[:, :, 8])
        lv_bt = const.tile([bpc, LH], f32, tag="lv_bt")
        nc.sync.dma_start(out=lv_bt, in_=obs[:, IH:IH + LH, 8])

        compat_cols = const.tile([LH, max(bpc, 2)], f32, tag="compat_cols")

        def ecopy(eng, out, in_):
            if eng is nc.scalar:
                nc.scalar.copy(out=out, in_=in_)
            else:
                eng.tensor_copy(out=out, in_=in_)

        def mm(out, lhsT, rhs, start, stop):
            nc.tensor.matmul(out, lhsT, rhs, start=start, stop=stop)

        def zero_pad_cols(t3):
            for k in range(2):
                nc.gpsimd.tensor_copy(out=t3[:, k, G:G + 1], in_=zcol[:, 0:1])

        def transpose_nat(dst3, src3, engs=(None, None)):
            """src3 [128,3,E] natural -> dst3 [128,2,304] transposed; one psum
            tile + one wide copy per e-chunk."""
            for k in range(2):
                idt = ident_r if src3.dtype == f32r else ident
                tpk = ps.tile([128, VN], src3.dtype, tag="ps", name=f"tpk{k}")
                for t in range(3):
                    g0, g1 = GC[t]
                    gs = g1 - g0
                    nc.tensor.transpose(tpk[:, g0:g1], src3[:gs, t, k * 128:(k + 1) * 128],
                                        idt[:gs, :gs])
                eng = engs[k] or (nc.scalar if k == 0 else nc.vector)
                ecopy(eng, dst3[:, k, :VN], tpk)

        def transpose_to(dst_sb, src_sb, copy_eng):
            """dst[j, i] = src[i, j] via PE; src [p, f] -> dst [f, p]."""
            p, f = src_sb.shape
            idt = ident_r if src_sb.dtype == f32r else ident
            tp = ps.tile([128, 128], src_sb.dtype, tag="ps")
            nc.tensor.transpose(tp[:f, :p], src_sb, idt[:p, :p])
            ecopy(copy_eng, dst_sb, tp[:f, :p])

        # ================= per batch element =================
        for bb in range(nbp if nbp is not None else bpc):
            b = bb % bpc
            xT = st.tile([9, 304], f32, tag="xT")
            nc.sync.dma_start(out=xT[:, :G], in_=obs[b].rearrange("g f -> f g"))

            # ---- embedding MLPs -> h [301, 256] natural (3 g-chunk tiles) ----
            z1 = ps.tile([32, G], f32, tag="ps")
            mm(z1[:, :IH], cw["wi1"], xT[:6, :IH], True, False)
            mm(z1[:, :IH], cw["b1r_i"], ones[:, :IH], False, True)
            mm(z1[:, IH:IH + LH], cw["wl1"], xT[:8, IH:IH + LH], True, False)
            mm(z1[:, IH:IH + LH], cw["b1r_l"], ones[:, :LH], False, True)
            mm(z1[:, IH + LH:], cw["wn1"], xT[:6, IH + LH:G], True, False)
            mm(z1[:, IH + LH:], cw["b1r_n"], ones[:, :1], False, True)

            if dbg and b == 0:
                z1c = st.tile([32, G], f32, tag="z1c")
                nc.gpsimd.tensor_copy(out=z1c, in_=z1) if False else nc.vector.tensor_copy(out=z1c, in_=z1)
                nc.sync.dma_start(out=dbg_z1[:], in_=z1c)
            small1 = st.tile([32, G], f32, tag="small1")
            nc.scalar.activation(small1, z1, AF.Identity, scale=0.01)
            lr = st.tile([32, G], f32r, tag="lr")
            nc.vector.tensor_tensor(out=lr, in0=z1, in1=small1, op=ALU.max)

            if dbg and b == 0:
                nc.sync.dma_start(out=dbg_lr[:], in_=lr.bitcast(f32))
            h0 = st.tile([128, 3, E], f32, tag="h0")
            segs = [(0, 128, "i"), (128, IH, "i"), (IH, 256, "l"), (256, 300, "l"), (300, 301, "n")]
            for si, (c0, c1, ty) in enumerate(segs):
                ti, r0 = c0 // 128, c0 % 128
                rows = c1 - c0
                hp = ps.tile([128, E], f32, tag="ps", name=f"hp{si}")
                mm(hp[:rows, :], lr[:, c0:c1], cw[f"w2_{ty}"], True, False)
                mm(hp[:rows, :], ones_r[:, :rows], cw[f"b2r_{ty}"], False, True)
                eng = nc.scalar if si % 2 == 0 else nc.vector
                if r0 % 32 == 0:
                    ecopy(eng, h0[r0:r0 + rows, ti, :], hp[:rows, :])
                else:
                    stg = st.tile([128, E], f32, tag="stg", name=f"stg{si}")
                    ecopy(eng, stg[:rows, :], hp[:rows, :])
                    nc.sync.dma_start(out=h0[r0:r0 + rows, ti, :], in_=stg[:rows, :])

            # initial transpose h -> hT [e, g]
            hT = st.tile([128, 2, 304], f32r, tag="hT")
            for t in range(3):
                g0, g1 = GC[t]
                gs = g1 - g0
                for k in range(2):
                    eng = nc.scalar if (t + k) % 2 == 0 else nc.vector
                    transpose_to(hT[:, k, g0:g1], h0[:gs, t, k * 128:(k + 1) * 128], eng)

            if dbg and b == 0:
                nc.sync.dma_start(out=dbg_h0[:], in_=h0)
            h_nat = h0
            # ================= encoder layers =================
            for l in range(NL):
                # q^T, k^T [e, g]
                qT = st.tile([128, 2, 304], f32r, tag="qT")
                kT = st.tile([128, 2, 304], f32r, tag="kT")
                for m in range(2):
                    qp = ps.tile([128, VN], f32, tag="ps")
                    for k in range(2):
                        mm(qp, cw[f"wq{l}"][:, k, m * 128:(m + 1) * 128], hT[:, k, :VN], k == 0, k == 1)
                    nc.scalar.copy(qT[:, m, :VN], qp)
                    kp = ps.tile([128, VN], f32, tag="ps")
                    for k in range(2):
                        mm(kp, cw[f"wk{l}"][:, k, m * 128:(m + 1) * 128], hT[:, k, :VN], k == 0, k == 1)
                    nc.vector.tensor_copy(kT[:, m, :VN], kp)

                # v natural [g, e]
                v = st.tile([128, 3, E], f32r, tag="v")
                for t in range(3):
                    g0, g1 = GC[t]
                    gs = g1 - g0
                    vp = ps.tile([128, E], f32, tag="ps")
                    for k in range(2):
                        mm(vp[:gs, :], hT[:, k, g0:g1], cw[f"wv{l}"][:, k, :], k == 0, k == 1)
                    nc.scalar.copy(v[:gs, t, :], vp[:gs, :])

                # scores + softmax (unnormalized exp; 1/rowsum deferred to o)
                a = st.tile([128, 3, 304], f32r, tag="a")
                rinv = st.tile([128, 3], f32, tag="rinv")
                for t in range(3):
                    g0, g1 = GC[t]
                    gs = g1 - g0
                    sp = ps.tile([128, VN], f32, tag="ps")
                    for k in range(2):
                        mm(sp[:gs, :], qT[:, k, g0:g1], kT[:, k, :VN], k == 0, k == 1)
                    rsum = st.tile([128, 1], f32, tag="rsum")
                    nc.scalar.activation(a[:gs, t, :G], sp[:gs, :G], AF.Exp,
                                         accum_out=rsum[:gs])
                    nc.vector.reciprocal(rinv[:gs, t:t + 1], rsum[:gs])

                if dbg and b == 0 and l == 0:
                    nc.sync.dma_start(out=dbg_a[:], in_=a.bitcast(f32))
                    nc.sync.dma_start(out=dbg_rinv[:], in_=rinv)
                # a^T [gk, gq] via PE transposes
                aT = st.tile([128, 3, 304], f32r, tag="aT")
                for tk in range(3):
                    k0, k1 = GC2[tk]
                    ks = k1 - k0
                    tpa = ps.tile([128, VN], f32r, tag="ps", name=f"tpa{tk}")
                    for tq in range(3):
                        q0, q1 = GC2[tq]
                        qs = q1 - q0
                        nc.tensor.transpose(tpa[:ks, q0:q1], a[:qs, tq, k0:k1],
                                            ident_r[:qs, :qs])
                    eng = (nc.scalar, nc.vector, nc.scalar)[tk]
                    ecopy(eng, aT[:ks, tk, :VN], tpa[:ks, :])

                # av^T [e, gq] = sum_gk v^T a^T
                avT = st.tile([128, 2, 304], f32r, tag="avT")
                for m in range(2):
                    ap_ = ps.tile([128, VN], f32, tag="ps")
                    for tk in range(3):
                        k0, k1 = GC[tk]
                        ks = k1 - k0
                        mm(ap_, v[:ks, tk, m * 128:(m + 1) * 128], aT[:ks, tk, :VN], tk == 0, tk == 2)
                    eng = nc.scalar if m == 0 else nc.vector
                    ecopy(eng, avT[:, m, :VN], ap_)

                # o = av @ Wo (natural), scale rows by 1/rowsum, +h, LN -> h1
                h1 = st.tile([128, 3, E], f32, tag="h1")
                x1 = st.tile([128, 3, E], f32, tag="x1")
                for t in range(3):
                    g0, g1 = GC[t]
                    gs = g1 - g0
                    op_ = ps.tile([128, E], f32, tag="ps")
                    for k in range(2):
                        mm(op_[:gs, :], avT[:, k, g0:g1], cw[f"wo{l}"][:, k, :], k == 0, k == 1)
                    nc.scalar.activation(x1[:gs, t, :], op_[:gs, :], AF.Identity,
                                         scale=rinv[:gs, t:t + 1])
                    nc.vector.tensor_add(x1[:gs, t, :], x1[:gs, t, :], h_nat[:gs, t, :])
                    _layernorm(nc, st, eps, x1, h1, t, gs, f32, AF, ALU)

                # h1 -> h1T
                h1T = st.tile([128, 2, 304], f32r, tag="h1T")
                transpose_nat(h1T, h1)
                zero_pad_cols(h1T)

                # FF: f1^T = relu(wf1^T h1^T + bf1) [f, g]
                f1 = st.tile([128, 4, 304], f32r, tag="f1")
                for m in range(4):
                    fp = ps.tile([128, VN], f32, tag="ps")
                    for k in range(2):
                        mm(fp, cw[f"wf1{l}"][:, k, m * 128:(m + 1) * 128], h1T[:, k, :VN], k == 0, k == 1)
                    nc.scalar.activation(f1[:, m, :G], fp[:, :G], AF.Relu,
                                         bias=cw[f"bf1{l}"][:, m:m + 1])

                # f2 = f1 @ wf2 + bf2 (natural), +h1, LN -> h2
                h2 = st.tile([128, 3, E], f32, tag="h2")
                x2 = st.tile([128, 3, E], f32, tag="x2")
                for t in range(3):
                    g0, g1 = GC[t]
                    gs = g1 - g0
                    f2p = ps.tile([128, E], f32, tag="ps")
                    for m in range(4):
                        mm(f2p[:gs, :], f1[:, m, g0:g1], cw[f"wf2{l}"][:, m, :], m == 0, False)
                    mm(f2p[:gs, :], ones_r[:, :gs], cw[f"bf2{l}"], False, True)
                    nc.vector.tensor_add(x2[:gs, t, :], f2p[:gs, :], h1[:gs, t, :])
                    _layernorm(nc, st, eps, x2, h2, t, gs, f32, AF, ALU)

                if dbg and b == 0:
                    nc.sync.dma_start(out=(dbg_h1 if l == 0 else dbg_h2)[:], in_=h2)
                h_nat = h2
                if l < NL - 1:
                    hT = st.tile([128, 2, 304], f32r, tag="hT")
                    transpose_nat(hT, h2)
                    zero_pad_cols(hT)

            # ================= decoder =================
            mcol = st.tile([128, 3], f32, tag="mcol")
            trans = st.tile([128, 3, E], f32, tag="trans")
            for t in range(3):
                g0, g1 = GC[t]
                gs = g1 - g0
                nc.sync.dma_start(out=mcol[:gs, t:t + 1], in_=obs[b, g0:g1, 8:9])
                nc.gpsimd.tensor_scalar_mul(trans[:gs, t, :], in0=h_nat[:gs, t, :],
                                            scalar1=mcol[:gs, t:t + 1])
            tT = st.tile([128, 2, 304], f32, tag="tT")
            transpose_nat(tT, trans)

            ge = st.tile([128, 2], f32, tag="ge")
            for k in range(2):
                nc.vector.reduce_sum(ge[:, k:k + 1], tT[:, k, :G], axis=AX.X)

            c_sb = st.tile([128, 2], f32, tag="c_sb")
            for m in range(2):
                cp = ps.tile([128, 1], f32, tag="ps")
                for k in range(2):
                    mm(cp, cw["mt"][:, k, m * 128:(m + 1) * 128], ge[:, k:k + 1], k == 0, k == 1)
                nc.scalar.copy(c_sb[:, m:m + 1], cp)

            cp2 = ps.tile([LH, 1], f32, tag="ps", name="cp2")
            for k in range(2):
                mm(cp2, tT[:, k, IH:IH + LH], c_sb[:, k:k + 1], k == 0, k == 1)
            nc.scalar.copy(compat_cols[:, b:b + 1], cp2)

        # ================= batched tail =================
        vl = const.tile([bpc, 1], f32, tag="vl")
        nc.vector.reduce_sum(vl, mask_bt, axis=AX.X)
        ivl = const.tile([bpc, 1], f32, tag="ivl")
        nc.vector.reciprocal(ivl, vl)

        ctp = ps.tile([128, LH], f32, tag="ps", name="ctp")
        nc.tensor.transpose(ctp[:bpc, :LH], compat_cols[:, :bpc], ident[:LH, :LH])
        compat_sb = const.tile([bpc, LH], f32, tag="compat_sb")
        nc.vector.tensor_copy(compat_sb, ctp[:bpc, :LH])
        th = const.tile([bpc, LH], f32, tag="th")
        nc.scalar.activation(th, compat_sb, AF.Tanh, scale=ivl)
        ex = const.tile([bpc, LH], f32, tag="ex")
        es = const.tile([bpc, 1], f32, tag="es")
        nc.scalar.activation(ex, th, AF.Exp, scale=CLIP, accum_out=es)
        er = const.tile([bpc, 1], f32, tag="er")
        nc.vector.reciprocal(er, es)
        pm = const.tile([bpc, LH], f32, tag="pm")
        nc.vector.tensor_scalar_mul(pm, in0=ex, scalar1=er)
        nc.vector.tensor_tensor(out=pm, in0=pm, in1=lv_bt, op=mybir.AluOpType.mult)
        nc.vector.tensor_scalar_add(pm, in0=pm, scalar1=1e-20)
        rs2 = const.tile([bpc, 1], f32, tag="rs2")
        nc.vector.reduce_sum(rs2, pm, axis=AX.X)
        rr2 = const.tile([bpc, 1], f32, tag="rr2")
        nc.vector.reciprocal(rr2, rs2)
        ob = const.tile([bpc, LH], f32, tag="ob")
        nc.vector.tensor_scalar_mul(ob, in0=pm, scalar1=rr2)
        nc.sync.dma_start(out=out_d[:], in_=ob)

    nc.finalize()
    return nc


def _layernorm(nc, st, eps, x, h_out, t, gs, f32, AF, ALU):
    """LN over free dim (256) of x[:gs, t, :] -> h_out[:gs, t, :]."""
    import concourse.mybir as mybir
    st6 = st.tile([128, 6], f32, tag="st6")
    nc.vector.bn_stats(out=st6[:gs], in_=x[:gs, t, :])
    mv = st.tile([128, 2], f32, tag="mv")
    nc.vector.bn_aggr(out=mv[:gs], in_=st6[:gs])
    std = st.tile([128, 1], f32, tag="std")
    nc.scalar.activation(std[:gs], mv[:gs, 1:2], AF.Sqrt, bias=eps[:gs])
    rstd = st.tile([128, 1], f32, tag="rstd")
    nc.vector.reciprocal(rstd[:gs], std[:gs])
    nc.vector.tensor_scalar(
        out=h_out[:gs, t, :], in0=x[:gs, t, :],
        scalar1=mv[:gs, 0:1], scalar2=rstd[:gs],
        op0=ALU.subtract, op1=ALU.mult,
    )


# ----------------------------------------------------------------------------
# public entry point
# ----------------------------------------------------------------------------
def kernel(**inputs):
    observation = np.asarray(inputs["observation"], np.float32)
    w = _prep_weights(inputs)

    from concourse.bass_utils import run_bass_kernel_spmd

    nc = _build(BPC)
    in_maps = []
    for i in range(NCORES):
        m = {"obs": np.ascontiguousarray(observation[i * BPC:(i + 1) * BPC])}
        m.update(w)
        in_maps.append(m)
    res = run_bass_kernel_spmd(nc, in_maps, list(range(NCORES)))
    out = np.concatenate([res.results[i]["out"] for i in range(NCORES)], axis=0)
    return out.astype(np.float32)
